# revision 1
# baseline (speedup 1.0000x reference)
"""Trainium2 Bass kernel for nn_CCL__69277822485245 (spectral conv via DCT/FFT).

Math: the reference's rFFT along W cancels into a circular 5-tap convolution,
and the DCT-II sandwich M @ diag(D[:,s]) @ D collapses into 5 dense 128x128
matrices G_s (precomputed on host). Per batch element:

    u_s[i, m, w] = sum_h G_s[m, h] x[i, h, w]                  (stage 1)
    out[o, m, n] = sum_{s,t,i} W[o,i,s,t] u_s[i, m, (n-t)%W] + bias[o]   (stage 2)

Sharding: data-parallel over batch B=8 across the 8 NeuronCores (1 each).

Layouts (per core):
  stage 1: per output column w, one matmul
      lhsT = xdup[h=128, di=128]    (x duplicated on the host so the output
                                     partition dim carries (d, i) pairs)
      rhs  = G^T[h=128, (s5, m64)]  (m in halves of 64 -> N=320; G s-order
                                     is [0,2,4,1,3] so each half's psum->u
                                     copy is a contiguous column slice)
      out  = psum[(d,i)=128, (sidx, m)]
      psum->SBUF casts split the halves: partitions 0-63 keep s={0,2,4}
      (slots 0..2), partitions 64-127 keep s={1,3} (slots 0..1), batched
      two w-columns per cast. SBUF u[(d,i), (slot, j, m)] -- j-major-of-m
      so stage-2 reads contiguous (j,m) runs.
  stage 2: for each t (same shift for both halves) and slot c:
      one K=128 matmul contracts (i, s=2c) on partitions 0-63 and
      (i, s=2c+1) on 64-127 simultaneously (c=2: K=64, s=4 only);
      15 sequential PSUM-accumulating passes, N = (j8, m64) = 512 contiguous.
      Bias added during the single per-block PSUM->SBUF evac (ScalarE).

DTYPE selects the matmul operand precision:
  "bf16": fastest (1 cyc/row + fast weight load), rel err ~ 3e-3
  "f32r": TF32-like (~2 cyc/row), rel err ~ 2e-4
  "f32" : exact fp32 (4 cyc/row), slowest
"""

import numpy as np

H = 128
W = 128
CI = 64
CO = 128
KH = 5
KW = 5
B = 8

MH = 64          # m-half processed per outer iteration
WB = 16          # w-block
HALO = 4         # extra back-columns for the t-shifts
WEXT = WB + HALO
NSLOT = 3        # s-slots per partition half (s = 2c + d)
JT = 8           # j-tile inside stage 2 (N = JT*MH = 512)

DTYPE = "bf16"

_PROG = None
_CONSTS = None
_RUN_OPTS = {}     # test harness may set e.g. {"trace": True, "trace_cores": [0]}
_LAST_RESULT = None


def _np_dt():
    if DTYPE == "bf16":
        import ml_dtypes
        return ml_dtypes.bfloat16
    return np.float32


def _build_consts():
    n = np.arange(H, dtype=np.float64)
    ang = np.pi * (2.0 * n[None, :] + 1.0) * n[:, None] / (2.0 * H)  # [k, h]
    D = 2.0 * np.cos(ang)
    wgt = np.where(n == 0, 0.5, 1.0)
    M = (np.cos(ang).T * wgt[None, :]) / (2.0 * H)                    # [m, k]
    G = np.stack([M @ (D[:, s:s + 1] * D) for s in range(KH)])        # [s, m, h]
    G = G[[0, 2, 4, 1, 3]]   # s-order so each half's psum->u copy is contiguous
    # rhs layout [h, (mh, sidx, ml)]: col = mh*320 + sidx*64 + ml
    GT = (G.transpose(2, 0, 1)                # [h, s, m]
            .reshape(H, KH, 2, MH)            # [h, s, mh, ml]
            .transpose(0, 2, 1, 3)            # [h, mh, s, ml]
            .reshape(H, KH * H))
    return np.ascontiguousarray(GT).astype(_np_dt())


def _build_program():
    import concourse.mybir as mybir
    import concourse.tile as tile
    from concourse import bacc

    f32 = mybir.dt.float32
    mmdt = {"bf16": mybir.dt.bfloat16,
            "f32r": mybir.dt.float32r,
            "f32": mybir.dt.float32}[DTYPE]

    nc = bacc.Bacc("TRN2", target_bir_lowering=False, debug=False,
                   enable_asserts=False, num_devices=B)
    x_d = nc.dram_tensor("x", [H, 2 * CI * W], mmdt, kind="ExternalInput").ap()
    g_d = nc.dram_tensor("g", [H, KH * H], mmdt, kind="ExternalInput").ap()
    w_d = nc.dram_tensor("wt", [128, KW * NSLOT * CO], mmdt,
                         kind="ExternalInput").ap()
    b_d = nc.dram_tensor("bias", [CO, 1], f32, kind="ExternalInput").ap()
    o_d = nc.dram_tensor("out", [CO, H, W], f32, kind="ExternalOutput").ap()

    with tile.TileContext(nc) as tc:
        with (
            tc.tile_pool(name="const", bufs=1) as cpool,
            tc.tile_pool(name="u", bufs=2) as upool,
            tc.tile_pool(name="oacc", bufs=1) as opool,
            tc.tile_pool(name="ps1", bufs=2, space="PSUM") as ps1,
            tc.tile_pool(name="ps2", bufs=2, space="PSUM") as ps2,
        ):
            xT = cpool.tile([H, 2 * CI * W], mmdt)
            nc.sync.dma_start(xT[:], x_d)
            gt = cpool.tile([H, KH * H], mmdt)
            nc.sync.dma_start(gt[:], g_d)
            wt = cpool.tile([128, KW * NSLOT * CO], mmdt)
            nc.sync.dma_start(wt[:], w_d)
            bt = cpool.tile([CO, 1], f32)
            nc.sync.dma_start(bt[:], b_d)

            import concourse.mybir as _mb

            x3 = xT[:].rearrange("p (di w) -> p di w", w=W)   # di = d*64+i

            def stage1(mh, blk):
                u = upool.tile([128, NSLOT * WEXT * MH], mmdt)
                u4 = u[:].rearrange("p (c j m) -> p c j m", c=NSLOT, j=WEXT)
                for j0 in range(0, WEXT, 2):
                    p1 = ps1.tile([128, 1024], f32)
                    for dj in range(2):
                        wg = (blk * WB - HALO + j0 + dj) % W
                        nc.tensor.matmul(p1[:, dj * 512:dj * 512 + KH * MH],
                                         x3[:, :, wg],
                                         gt[:, mh * KH * MH:(mh + 1) * KH * MH],
                                         start=True, stop=True)
                    pv = p1[:].rearrange("p (j s m) -> p j s m", j=2, s=8)
                    # psum s-order [0,2,4,1,3]: half0 cols 0:192, half1 192:320
                    nc.vector.tensor_copy(
                        u4[0:64, :, j0:j0 + 2, :].transpose([0, 2, 1, 3]),
                        pv[0:64, :, 0:3, :])
                    nc.vector.tensor_copy(
                        u4[64:128, 0:2, j0:j0 + 2, :].transpose([0, 2, 1, 3]),
                        pv[64:128, :, 3:5, :])
                return u4

            def stage2(u4, oa3, blk):
                p2 = ps2.tile([128, 2 * 512], f32)
                for t in range(KW):
                    for c in range(NSLOT):
                        kk = 128 if c < 2 else 64
                        lhsT2 = wt[0:kk, (t * NSLOT + c) * CO:
                                   (t * NSLOT + c + 1) * CO]
                        start = (t == 0 and c == 0)
                        stop = (t == KW - 1 and c == NSLOT - 1)
                        for jt in range(WB // JT):
                            # contiguous (j8, m64) = 512 elems
                            rhs2 = u4[0:kk, c,
                                      HALO - t + jt * JT:
                                      HALO - t + (jt + 1) * JT, :]
                            nc.tensor.matmul(
                                p2[:, jt * 512:(jt + 1) * 512], lhsT2, rhs2,
                                start=start, stop=stop)
                p23 = p2[:].rearrange("p (jt j m) -> p jt j m", jt=2, j=JT)
                nc.scalar.activation(
                    oa3[:, :, blk * WB:(blk + 1) * WB]
                        .rearrange("p m (jt j) -> p m jt j", jt=2),
                    p23[:].transpose([0, 3, 1, 2]),
                    _mb.ActivationFunctionType.Identity, bias=bt[:])

            # software pipeline: stage1(k+1) is emitted before stage2(k) so the
            # in-order PE queue fills cast-wait gaps with ready matmul work.
            NBLK = W // WB
            tiles = [(mh, blk) for mh in range(2) for blk in range(NBLK)]
            oaccs = {}
            for mh in range(2):
                oacc = opool.tile([CO, MH * W], f32, tag=f"oacc{mh}")
                oaccs[mh] = oacc[:].rearrange("p (m w) -> p m w", w=W)
            pend = stage1(*tiles[0])
            for k, (mh, blk) in enumerate(tiles):
                nxt = stage1(*tiles[k + 1]) if k + 1 < len(tiles) else None
                stage2(pend, oaccs[mh], blk)
                if blk == NBLK - 1:
                    nc.sync.dma_start(o_d[:, mh * MH:(mh + 1) * MH, :], oaccs[mh])
                pend = nxt
    nc.compile()
    return nc


def _get_prog():
    global _PROG
    if _PROG is None:
        _PROG = _build_program()
    return _PROG


def _build_wstack(weight):
    # wst[(d,i), (t, c, o)]: d=0 -> s=2c ; d=1 -> s=2c+1 (c<2), zeros for c=2
    wst = np.zeros((128, KW * NSLOT * CO), np.float32)
    for t in range(KW):
        for c in range(NSLOT):
            col = (t * NSLOT + c) * CO
            wst[0:64, col:col + CO] = weight[:, :, 2 * c, t].T
            if c < 2:
                wst[64:128, col:col + CO] = weight[:, :, 2 * c + 1, t].T
    return np.ascontiguousarray(wst).astype(_np_dt())


def kernel(x, weight, bias):
    from concourse.bass_utils import run_bass_kernel_spmd

    global _CONSTS
    if _CONSTS is None:
        _CONSTS = _build_consts()
    GT = _CONSTS

    x = np.ascontiguousarray(np.asarray(x, dtype=np.float32))
    weight = np.ascontiguousarray(np.asarray(weight, dtype=np.float32))
    bias = np.ascontiguousarray(np.asarray(bias, dtype=np.float32))

    wst = _build_wstack(weight)
    b2 = np.ascontiguousarray(bias.reshape(CO, 1))

    in_maps = []
    for b in range(B):
        xt = np.ascontiguousarray(x[b].transpose(1, 0, 2)).reshape(H, CI * W)
        xdup = np.ascontiguousarray(
            np.concatenate([xt, xt], axis=1)).astype(_np_dt())
        in_maps.append({"x": xdup, "g": GT, "wt": wst, "bias": b2})

    res = run_bass_kernel_spmd(_get_prog(), in_maps, core_ids=list(range(B)),
                               **_RUN_OPTS)
    global _LAST_RESULT
    _LAST_RESULT = res
    out = np.stack([res.results[b]["out"] for b in range(B)], axis=0)
    return np.ascontiguousarray(out.astype(np.float32))



# revision 3
# speedup vs baseline: 1.1398x; 1.1398x over previous
"""Trainium2 Bass kernel for nn_CCL__69277822485245 (spectral conv via DCT/FFT).

Math: the reference's rFFT along W cancels into a circular 5-tap convolution,
and the DCT-II sandwich M @ diag(D[:,s]) @ D collapses into 5 dense 128x128
matrices G_s (precomputed on host). Per batch element:

    u_s[i, m, w] = sum_h G_s[m, h] x[i, h, w]                  (stage 1)
    out[o, m, n] = sum_{s,t,i} W[o,i,s,t] u_s[i, m, (n-t)%W] + bias[o]

Sharding: data-parallel over batch B=8 across the 8 NeuronCores (1 each).

v2 layout — w-parity packing (no duplication, no w-halo in stage 1):
  stage 1: lhsT = x2[h=128, (w-pair jp -> 128 cols: w=2jp i0..63, w=2jp+1
      i0..63)] (stationary, one load per jp), rhs = gt[h, (mh, s, m)] N=320.
      psum[(wp,i), (s,m)] -> one straight (non-transposing) copy per (jp,mh)
      into u[(wp,i), s, HALO+jp, m]; jp 62,63 also copied to the front halo
      slots (circular W).
  stage 2: output n split by parity p; kernel taps t pair across partition
      halves by w-parity of n-t. Per (s,p): two K=128 pairs + one K=64 solo,
      each a jp-offset slice of u. 15 accumulating matmuls per psum chunk,
      chunk = [o=128, (jp=64, m=8)] so finished output is contiguous per
      m-row -> efficient streaming DMA out per 8-m block.

DTYPE "bf16": 1 cyc/row matmuls, rel err ~ 3e-3 (gate 2e-2).
"""

import numpy as np

H = 128
W = 128
CI = 64
CO = 128
KH = 5
KW = 5
B = 8

MH = 64          # m-half processed per outer iteration
JP = W // 2      # 64 w-pairs
HALO = 2         # front jp-halo (circular W wrap for t-shifts)
JX = HALO + JP   # 66

DTYPE = "bf16"

_PROG = None
_CONSTS = None
_RUN_OPTS = {}     # test harness may set e.g. {"trace": True, "trace_cores": [0]}
_LAST_RESULT = None

# stage-2 slot groups per parity: (s, gi) -> (jp_offset, kbase, kk)
#   p=0: gi0 = (t2|t1) off -1, gi1 = (t4|t3) off -2, gi2 = (t0|--) off 0 K=64 lo
#   p=1: gi0 = (t1|t0) off  0, gi1 = (t3|t2) off -1, gi2 = (--|t4) off -2 K=64 hi
_GROUPS = {
    0: [(-1, 0, 128), (-2, 0, 128), (0, 0, 64)],
    1: [(0, 0, 128), (-1, 0, 128), (-2, 64, 64)],
}


def _np_dt():
    if DTYPE == "bf16":
        import ml_dtypes
        return ml_dtypes.bfloat16
    return np.float32


def _build_consts():
    n = np.arange(H, dtype=np.float64)
    ang = np.pi * (2.0 * n[None, :] + 1.0) * n[:, None] / (2.0 * H)  # [k, h]
    D = 2.0 * np.cos(ang)
    wgt = np.where(n == 0, 0.5, 1.0)
    M = (np.cos(ang).T * wgt[None, :]) / (2.0 * H)                    # [m, k]
    G = np.stack([M @ (D[:, s:s + 1] * D) for s in range(KH)])        # [s, m, h]
    # gt layout [h, (mh, s, m)]: col = mh*320 + s*64 + ml
    GT = (G.transpose(2, 0, 1)                # [h, s, m]
            .reshape(H, KH, 2, MH)            # [h, s, mh, ml]
            .transpose(0, 2, 1, 3)            # [h, mh, s, ml]
            .reshape(H, KH * H))
    return np.ascontiguousarray(GT).astype(_np_dt())


def _build_wstack(weight):
    # wst[(d,i), (p, s, gi, o)]: see _GROUPS; d = w-parity partition half
    wst = np.zeros((128, 2 * KH * 3 * CO), np.float32)
    col = 0
    for p in range(2):
        for s in range(KH):
            Wl = weight[:, :, s, :]          # [o, i, t]
            if p == 0:
                pairs = [(2, 1), (4, 3)]     # (lower half t, upper half t)
                solo = (0, 0)                # (t, kbase)
            else:
                pairs = [(1, 0), (3, 2)]
                solo = (4, 64)
            for tl, tu in pairs:
                wst[0:64, col:col + CO] = Wl[:, :, tl].T
                wst[64:128, col:col + CO] = Wl[:, :, tu].T
                col += CO
            t, kb = solo
            wst[kb:kb + 64, col:col + CO] = Wl[:, :, t].T
            col += CO
    return np.ascontiguousarray(wst).astype(_np_dt())


def _build_program():
    import concourse.mybir as mybir
    import concourse.tile as tile
    from concourse import bacc

    f32 = mybir.dt.float32
    mmdt = {"bf16": mybir.dt.bfloat16,
            "f32r": mybir.dt.float32r,
            "f32": mybir.dt.float32}[DTYPE]

    nc = bacc.Bacc("TRN2", target_bir_lowering=False, debug=False,
                   enable_asserts=False, num_devices=B)
    x_d = nc.dram_tensor("x", [H, W * CI], mmdt, kind="ExternalInput").ap()
    g_d = nc.dram_tensor("g", [H, KH * H], mmdt, kind="ExternalInput").ap()
    w_d = nc.dram_tensor("wt", [128, 2 * KH * 3 * CO], mmdt,
                         kind="ExternalInput").ap()
    b_d = nc.dram_tensor("bias", [CO, 1], f32, kind="ExternalInput").ap()
    o_d = nc.dram_tensor("out", [CO, H, W], f32, kind="ExternalOutput").ap()

    with tile.TileContext(nc) as tc:
        with (
            tc.tile_pool(name="const", bufs=1) as cpool,
            tc.tile_pool(name="u", bufs=1) as upool,
            tc.tile_pool(name="oacc", bufs=1) as opool,
            tc.tile_pool(name="ps1", bufs=2, space="PSUM") as ps1,
            tc.tile_pool(name="ps2", bufs=4, space="PSUM") as ps2,
        ):
            xt = cpool.tile([H, W * CI], mmdt)
            # chunked along w so stage 1 can start on the first quarter
            for c in range(4):
                nc.sync.dma_start(xt[:, c * 2048:(c + 1) * 2048],
                                  x_d[:, c * 2048:(c + 1) * 2048])
            gt = cpool.tile([H, KH * H], mmdt)
            nc.sync.dma_start(gt[:], g_d)
            wt = cpool.tile([128, 2 * KH * 3 * CO], mmdt)
            nc.sync.dma_start(wt[:], w_d)
            bt = cpool.tile([CO, 1], f32)
            nc.sync.dma_start(bt[:], b_d)

            import concourse.mybir as _mb

            def stage1(mh):
                u = upool.tile([128, KH * JX * MH], mmdt, tag=f"u{mh}")
                u4 = u[:].rearrange("p (s j m) -> p s j m", s=KH, j=JX)
                for jp in range(JP):
                    p1 = ps1.tile([128, KH * MH], f32)
                    nc.tensor.matmul(p1[:],
                                     xt[:, jp * 128:(jp + 1) * 128],
                                     gt[:, mh * KH * MH:(mh + 1) * KH * MH],
                                     start=True, stop=True)
                    pv = p1[:].rearrange("p (s m) -> p s m", s=KH)
                    nc.vector.tensor_copy(u4[:, :, HALO + jp, :], pv)
                    if jp >= JP - HALO:   # circular wrap into front halo
                        nc.vector.tensor_copy(u4[:, :, jp - (JP - HALO), :], pv)
                return u4

            def stage2(u4, mh):
                oacc = opool.tile([CO, MH * W], f32, tag=f"oacc{mh}")
                # [o, m, jn-pair, parity]
                oa4 = oacc[:].rearrange("p (m j q) -> p m j q", m=MH, q=2)
                for mc in range(8):          # 8-m chunk
                    for p in range(2):
                        p2 = ps2.tile([128, JP * 8], f32)
                        for gi in range(3 * KH):
                            s, g = divmod(gi, 3)
                            off, kb, kk = _GROUPS[p][g]
                            rhs = u4[kb:kb + kk, s,
                                     HALO + off:HALO + off + JP,
                                     mc * 8:(mc + 1) * 8]
                            gb = p * 15 + s * 3 + g
                            nc.tensor.matmul(
                                p2[:], wt[kb:kb + kk, gb * CO:(gb + 1) * CO],
                                rhs, start=(gi == 0), stop=(gi == 3 * KH - 1))
                        p23 = p2[:].rearrange("p (j m) -> p j m", j=JP)
                        nc.scalar.activation(
                            oa4[:, mc * 8:(mc + 1) * 8, :, p],
                            p23[:].transpose([0, 2, 1]),
                            _mb.ActivationFunctionType.Identity, bias=bt[:])
                    # both parities of this m-chunk done -> stream out
                    nc.sync.dma_start(
                        o_d[:, mh * MH + mc * 8:mh * MH + (mc + 1) * 8, :],
                        oacc[:, mc * 8 * W:(mc + 1) * 8 * W])

            u0 = stage1(0)
            u1 = stage1(1)
            stage2(u0, 0)
            stage2(u1, 1)
    nc.compile()
    return nc


def _get_prog():
    global _PROG
    if _PROG is None:
        _PROG = _build_program()
    return _PROG


def kernel(x, weight, bias):
    from concourse.bass_utils import run_bass_kernel_spmd

    global _CONSTS
    if _CONSTS is None:
        _CONSTS = _build_consts()
    GT = _CONSTS

    x = np.ascontiguousarray(np.asarray(x, dtype=np.float32))
    weight = np.ascontiguousarray(np.asarray(weight, dtype=np.float32))
    bias = np.ascontiguousarray(np.asarray(bias, dtype=np.float32))

    wst = _build_wstack(weight)
    b2 = np.ascontiguousarray(bias.reshape(CO, 1))

    in_maps = []
    for b in range(B):
        # x2[h, (w, i)]
        x2 = np.ascontiguousarray(
            x[b].transpose(1, 2, 0).reshape(H, W * CI)).astype(_np_dt())
        in_maps.append({"x": x2, "g": GT, "wt": wst, "bias": b2})

    res = run_bass_kernel_spmd(_get_prog(), in_maps, core_ids=list(range(B)),
                               **_RUN_OPTS)
    global _LAST_RESULT
    _LAST_RESULT = res
    out = np.stack([res.results[b]["out"] for b in range(B)], axis=0)
    return np.ascontiguousarray(out.astype(np.float32))
